# revision 1
# baseline (speedup 1.0000x reference)
"""Trainium2 Bass kernel for nn_MultiHeadAttention_3839700762945.

Full-shape contract: kernel(**inputs) takes the unsharded numpy inputs and
returns the full [4, 2048, 1024] output.

Sharding (8 cores): core c handles (batch b = c//2, head-half = c%2).
Each core computes q/k/v projections for its 8 heads (512 of the 1024 dim
columns) over the full sequence, runs attention for those heads, and emits a
partial output projection  OT_half.T @ Wo[half]  of shape [2048, 1024].
Host combines: out[b] = partial[2b] + partial[2b+1] + bo.  No collectives.

On-chip dataflow (per core, all matmuls in float32r = full-rate TF32-like):
  - Q/K/V are transposed on the PE (128x128 identity-transpose tiles) into
    [dim, seq] layout, rounded to f32r on eviction.
  - qT/kT are produced transposed ([d, s]) via lhsT=W chunks; v is produced
    natural ([s, d], bf16) with a ones-column appended for softmax row sums.
  - scoresT[sk, sq] = kT_h^T qT_h per head; exp via ScalarE (scale=1/8 folded
    in, no max-subtraction: scores ~ N(0,1), fp32 exp is safe), bf16 P tiles.
  - AV: psum[0:65] = [v_h | 1]^T @ P accumulated over sk; row 64 = softmax
    denominator.  Normalization via reciprocal + PE outer-product broadcast.
  - output projection from the transposed attention output (natural layout
    for lhsT) with Wo natural as moving operand.
"""

import sys

for _p in ("/opt/trn_rl_repo", "/opt/pypackages"):
    if _p not in sys.path:
        sys.path.insert(0, _p)

import numpy as np

import concourse.bass as bass
import concourse.mybir as mybir
import concourse.tile as tile
import concourse.bacc as bacc
from concourse import masks
from concourse.bass_utils import run_bass_kernel_spmd

F32 = mybir.dt.float32
F32R = mybir.dt.float32r
BF16 = mybir.dt.bfloat16
AF = mybir.ActivationFunctionType

B, S, DIM = 4, 2048, 1024
DH = 512          # dim columns per core (8 heads x 64)
NH = 8            # heads per core
HD = 64
P = 128
NKC = DIM // P    # 8 contraction chunks for projections
NMC = DH // P     # 4 output-dim chunks
NSK = S // P      # 16 sk chunks
BW = 256          # transpose/projection block width (seq cols per block)
NBLK = S // BW    # 8 blocks
SQT = 512         # attention query tile
NSQT = S // SQT   # 4
EG = 2            # exp group: sk chunks per ScalarE activation op
INV_SQRT_HD = 0.125


def _emit_input_phase(nc, pools, Xdram, Wdram, Bdram, kind, kT=None, vsb=None):
    """Transpose one input to [dim, seq] blocks and project it.

    kind: 'kq' -> write transposed projection into kT ([128, 4, 2048] f32r),
          'v'  -> write natural projection into vsb ([128, 16, 8, 66] bf16).
    """
    (pc, p2, p3, p4, ps_pp, ps_sc, ps_av) = pools

    # load + round weights (two halves through an 8KB staging tile)
    wsb = pc.tile([P, NKC, DH], F32R, tag="wproj")
    wview = Wdram.ap().rearrange("(kc p) d -> p kc d", p=P)
    for hw in range(2):
        wst = pc.tile([P, NKC // 2, DH], F32, tag="wstage")
        nc.sync.dma_start(wst[:], wview[:, hw * 4:(hw + 1) * 4, :])
        nc.vector.tensor_copy(wsb[:, hw * 4:(hw + 1) * 4, :], wst[:])

    # load + round bias row [1, 512]
    brow = pc.tile([1, DH], F32R, tag=f"brow_{kind}_{'v' if vsb is not None else 'kq'}")
    bst = pc.tile([1, DH], F32, tag="bstage")
    nc.sync.dma_start(bst[:], Bdram.ap())
    nc.vector.tensor_copy(brow[:], bst[:])

    ident = pools_consts["ident"]
    ones = pools_consts["ones"]
    Xap = Xdram.ap()

    for blk in range(NBLK):
        xts = p2.tile([P, NKC, BW], F32R, tag="xt")
        for j in range(2):
            xn = p2.tile([P, DIM], F32, tag="xnat")
            r0 = (blk * 2 + j) * P
            nc.sync.dma_start(xn[:], Xap[r0:r0 + P, :])
            for kq in range(2):
                pst = ps_pp.tile([P, 4, P], F32, tag="pp")
                for ki in range(4):
                    k = kq * 4 + ki
                    nc.tensor.transpose(
                        pst[:, ki, :], xn[:, k * P:(k + 1) * P], ident[:]
                    )
                nc.vector.tensor_copy(
                    xts[:, kq * 4:(kq + 1) * 4, j * P:(j + 1) * P], pst[:]
                )

        if kind == "kq":
            # out_T[d, sk] block: lhsT = W chunk (natural), rhs = X^T block
            for m in range(NMC):
                psp = ps_pp.tile([P, BW], F32, tag="pp")
                for k in range(NKC):
                    nc.tensor.matmul(
                        psp[:],
                        wsb[:, k, m * P:(m + 1) * P],
                        xts[:, k, :],
                        start=(k == 0),
                        stop=False,
                    )
                nc.tensor.matmul(
                    psp[:],
                    brow[0:1, m * P:(m + 1) * P],
                    ones[0:1, 0:BW],
                    start=False,
                    stop=True,
                )
                nc.vector.tensor_copy(
                    kT[:, m, blk * BW:(blk + 1) * BW], psp[:]
                )
        else:
            # v natural [sk, d]: lhsT = X^T chunk, rhs = W (moving, N=512)
            for j in range(2):
                c = blk * 2 + j
                psv = ps_pp.tile([P, DH], F32, tag="pp")
                for k in range(NKC):
                    nc.tensor.matmul(
                        psv[:],
                        xts[:, k, j * P:(j + 1) * P],
                        wsb[:, k, :],
                        start=(k == 0),
                        stop=False,
                    )
                nc.tensor.matmul(
                    psv[:],
                    ones[0:1, 0:P],
                    brow[0:1, :],
                    start=False,
                    stop=True,
                )
                nc.vector.tensor_copy(
                    vsb[:, c, :, 0:HD],
                    psv[:].rearrange("p (h d) -> p h d", h=NH),
                )


pools_consts = {}


def build_nc(reps: int = 1, mode: str = "full"):
    """Build the per-core Bass program (SPMD: all cores run this)."""
    nc = bacc.Bacc("TRN2", target_bir_lowering=False, debug=False, num_devices=8)

    XQ = nc.dram_tensor("XQ", (S, DIM), F32, kind="ExternalInput")
    XK = nc.dram_tensor("XK", (S, DIM), F32, kind="ExternalInput")
    XV = nc.dram_tensor("XV", (S, DIM), F32, kind="ExternalInput")
    WQ = nc.dram_tensor("WQ", (DIM, DH), F32, kind="ExternalInput")
    WK = nc.dram_tensor("WK", (DIM, DH), F32, kind="ExternalInput")
    WV = nc.dram_tensor("WV", (DIM, DH), F32, kind="ExternalInput")
    WO = nc.dram_tensor("WO", (DH, DIM), F32, kind="ExternalInput")
    BQ = nc.dram_tensor("BQ", (1, DH), F32, kind="ExternalInput")
    BK = nc.dram_tensor("BK", (1, DH), F32, kind="ExternalInput")
    BV = nc.dram_tensor("BV", (1, DH), F32, kind="ExternalInput")
    OUT = nc.dram_tensor("OUT", (S, DIM), F32, kind="ExternalOutput")

    with tile.TileContext(nc) as tc:
        with (
            tc.tile_pool(name="persist", bufs=1) as pc,
            tc.tile_pool(name="dbuf", bufs=2) as p2,
            tc.tile_pool(name="tri", bufs=3) as p3,
            tc.tile_pool(name="quad", bufs=4) as p4,
            tc.tile_pool(name="ps_pp", bufs=2, space="PSUM") as ps_pp,
            tc.tile_pool(name="ps_sc", bufs=2, space="PSUM") as ps_sc,
            tc.tile_pool(name="ps_av", bufs=2, space="PSUM") as ps_av,
        ):
            pools = (pc, p2, p3, p4, ps_pp, ps_sc, ps_av)

            # constants
            ident = pc.tile([P, P], F32, tag="ident")
            masks.make_identity(nc, ident[:])
            ones_f32 = pc.tile([1, BW], F32, tag="ones_st")
            nc.vector.memset(ones_f32[:], 1.0)
            ones = pc.tile([1, BW], F32R, tag="ones")
            nc.vector.tensor_copy(ones[:], ones_f32[:])
            pools_consts["ident"] = ident
            pools_consts["ones"] = ones

            attn_reps = reps if "repattn" in mode else 1
            outer_reps = 1 if "repattn" in mode else reps
            for _rep in range(outer_reps):
                # persistent per-rep tensors
                kT = pc.tile([P, NMC, S], F32R, tag="kT")
                qT = pc.tile([P, NMC, S], F32R, tag="qT")
                vsb = pc.tile([P, NSK, NH, HD + 2], BF16, tag="vsb")
                nc.vector.memset(vsb[:, :, :, HD:HD + 1], 1.0)

                wo_sb = pc.tile([P, NMC, DIM], F32R, tag="wo")
                woview = WO.ap().rearrange("(kc p) d -> p kc d", p=P)
                for hw in range(2):
                    wst = pc.tile([P, 2, DIM], F32, tag="wstage")
                    nc.sync.dma_start(wst[:], woview[:, hw * 2:(hw + 1) * 2, :])
                    nc.vector.tensor_copy(wo_sb[:, hw * 2:(hw + 1) * 2, :], wst[:])

                _emit_input_phase(nc, pools, XK, WK, BK, "kq", kT=kT)
                _emit_input_phase(nc, pools, XV, WV, BV, "v", vsb=vsb)

                for sqt in range(NSQT):
                    _emit_q_blocks(nc, pools, XQ, WQ, BQ, qT, sqt)
                if mode != "phase_a":
                    for _ar in range(attn_reps):
                        for sqt in range(NSQT):
                            _emit_attention(nc, pools, kT, qT, vsb, wo_sb, OUT,
                                            sqt, mode)
                if mode == "phase_a":
                    # consume kT/qT/vsb so DCE keeps phase A
                    for m in range(NMC):
                        nc.sync.dma_start(
                            OUT.ap()[m * P:(m + 1) * P, 0:S // 2],
                            kT[:, m, 0:S // 2].bitcast(F32))
                        nc.sync.dma_start(
                            OUT.ap()[(4 + m) * P:(5 + m) * P, 0:S // 2],
                            qT[:, m, 0:S // 2].bitcast(F32))
                    vtmp = p2.tile([P, 512], F32, tag="vtmp")
                    nc.vector.tensor_copy(
                        vtmp[:],
                        vsb[:].rearrange("p a b c -> p (a b c)").bitcast(F32)[:, 0:512])
                    nc.sync.dma_start(OUT.ap()[1024:1024 + P, 0:512], vtmp[:])

    nc.compile()
    return nc


def _emit_q_blocks(nc, pools, XQ, WQ, BQ, qT, sqt):
    """Emit transpose+projection for the two 256-col Q blocks feeding sq tile
    `sqt` (cols sqt*512 .. sqt*512+512)."""
    (pc, p2, p3, p4, ps_pp, ps_sc, ps_av) = pools
    ident = pools_consts["ident"]
    ones = pools_consts["ones"]

    if sqt == 0:
        # weights + bias once
        wsb = pc.tile([P, NKC, DH], F32R, tag="wproj")
        wview = WQ.ap().rearrange("(kc p) d -> p kc d", p=P)
        for hw in range(2):
            wst = pc.tile([P, NKC // 2, DH], F32, tag="wstage")
            nc.sync.dma_start(wst[:], wview[:, hw * 4:(hw + 1) * 4, :])
            nc.vector.tensor_copy(wsb[:, hw * 4:(hw + 1) * 4, :], wst[:])
        brow = pc.tile([1, DH], F32R, tag="brow_q")
        bst = pc.tile([1, DH], F32, tag="bstage")
        nc.sync.dma_start(bst[:], BQ.ap())
        nc.vector.tensor_copy(brow[:], bst[:])
        pools_consts["wq_sb"] = wsb
        pools_consts["bq_row"] = brow
    wsb = pools_consts["wq_sb"]
    brow = pools_consts["bq_row"]
    Xap = XQ.ap()

    for blk in (2 * sqt, 2 * sqt + 1):
        xts = p2.tile([P, NKC, BW], F32R, tag="xt")
        for j in range(2):
            xn = p2.tile([P, DIM], F32, tag="xnat")
            r0 = (blk * 2 + j) * P
            nc.sync.dma_start(xn[:], Xap[r0:r0 + P, :])
            for kq in range(2):
                pst = ps_pp.tile([P, 4, P], F32, tag="pp")
                for ki in range(4):
                    k = kq * 4 + ki
                    nc.tensor.transpose(
                        pst[:, ki, :], xn[:, k * P:(k + 1) * P], ident[:]
                    )
                nc.vector.tensor_copy(
                    xts[:, kq * 4:(kq + 1) * 4, j * P:(j + 1) * P], pst[:]
                )
        for m in range(NMC):
            psp = ps_pp.tile([P, BW], F32, tag="pp")
            for k in range(NKC):
                nc.tensor.matmul(
                    psp[:],
                    wsb[:, k, m * P:(m + 1) * P],
                    xts[:, k, :],
                    start=(k == 0),
                    stop=False,
                )
            nc.tensor.matmul(
                psp[:],
                brow[0:1, m * P:(m + 1) * P],
                ones[0:1, 0:BW],
                start=False,
                stop=True,
            )
            nc.vector.tensor_copy(qT[:, m, blk * BW:(blk + 1) * BW], psp[:])


def _emit_attention(nc, pools, kT, qT, vsb, wo_sb, OUT, sqt, mode="full"):
    (pc, p2, p3, p4, ps_pp, ps_sc, ps_av) = pools
    ones = pools_consts["ones"]
    sq0 = sqt * SQT

    ot = p2.tile([P, NMC, SQT], F32R, tag="ot", bufs=1)
    rshs = {}
    NG = NSK // EG
    total = NH * NG
    psavs = {}
    ptts = {}

    # software pipeline: scores/exp for group idx, AV for group idx-1 —
    # keeps ScalarE (exp) saturated; PE never sits between exp and AV.
    for idx in range(total + 1):
        if idx < total:
            h, g = divmod(idx, NG)
            base = (h % 2) * HD
            mch = h // 2
            pss = ps_sc.tile([P, EG, SQT], F32, tag="sc")
            for ci in range(EG):
                c = g * EG + ci
                nc.tensor.matmul(
                    pss[:, ci, :],
                    kT[base:base + HD, mch, c * P:(c + 1) * P],
                    qT[base:base + HD, mch, sq0:sq0 + SQT],
                    start=True,
                    stop=True,
                )
            ptt = p4.tile([P, EG, SQT], BF16, tag="pt", bufs=3)
            if "noexp" in mode:
                nc.vector.tensor_copy(ptt[:], pss[:])
            else:
                nc.scalar.activation(ptt[:], pss[:], AF.Exp, scale=INV_SQRT_HD)
            ptts[idx] = ptt
        if idx >= 1:
            h2, g2 = divmod(idx - 1, NG)
            if g2 == 0:
                psavs[h2] = ps_av.tile([P, SQT], F32, tag="av", name="psav")
            ptt2 = ptts.pop(idx - 1)
            for ci in range(EG):
                c = g2 * EG + ci
                nc.tensor.matmul(
                    psavs[h2][0:HD + 1, :],
                    vsb[:, c, h2, 0:HD + 1],
                    ptt2[:, ci, :],
                    start=(c == 0),
                    stop=(c == NSK - 1),
                )
            if g2 == NG - 1:
                base2 = (h2 % 2) * HD
                mch2 = h2 // 2
                psav = psavs.pop(h2)
                rsh = p2.tile([1, SQT], F32R, tag="rsh", name="rsh")
                nc.vector.tensor_copy(rsh[:], psav[HD:HD + 1, :])
                # broadcast the rowsum down 64 partitions via a K=1 PE
                # outer product, reciprocal on DVE, multiply from PSUM
                psb = ps_pp.tile([P, SQT], F32, tag="pp", name="psb")
                nc.tensor.matmul(psb[0:HD, :], ones[0:1, 0:HD],
                                 rsh[:], start=True, stop=True)
                bcs = p2.tile([HD, SQT], F32, tag="bc", name="bcs")
                nc.vector.tensor_copy(bcs[:], psb[0:HD, :])
                if "normcopy" in mode:
                    nc.vector.tensor_mul(ot[base2:base2 + HD, mch2, :],
                                         psav[0:HD, :], bcs[:])
                else:
                    rcb = p2.tile([HD, SQT], F32, tag="rcb", name="rcb")
                    nc.vector.reciprocal_approx_fast(rcb[:], bcs[:])
                    nc.vector.tensor_mul(ot[base2:base2 + HD, mch2, :],
                                         psav[0:HD, :], rcb[:])

    # output projection for this sq tile: out[sq, :] = ot^T @ Wo (partial)
    for m in range(NMC):
        ostg = p2.tile([P, 2, DH], F32, tag="ostg")
        for n2 in range(2):
            pso = ps_pp.tile([P, DH], F32, tag="pp")
            for k in range(NMC):
                nc.tensor.matmul(
                    pso[:],
                    ot[:, k, m * P:(m + 1) * P],
                    wo_sb[:, k, n2 * DH:(n2 + 1) * DH],
                    start=(k == 0),
                    stop=(k == NMC - 1),
                )
            nc.vector.tensor_copy(ostg[:, n2, :], pso[:])
        r0 = sq0 + m * P
        nc.sync.dma_start(
            OUT.ap()[r0:r0 + P, :].rearrange("p (n d) -> p n d", n=2), ostg[:]
        )


_cached = {}


def _get_nc(reps: int = 1, mode: str = "full"):
    key = (reps, mode)
    if key not in _cached:
        _cached[key] = build_nc(reps, mode)
    return _cached[key]


def make_in_maps(Q, K, V, Wq, bq, Wk, bk, Wv, bv, Wo, bo):
    asf = lambda x: np.ascontiguousarray(np.asarray(x, dtype=np.float32))
    in_maps = []
    for c in range(8):
        b, half = divmod(c, 2)
        sl = slice(half * DH, (half + 1) * DH)
        in_maps.append({
            "XQ": asf(Q[b]),
            "XK": asf(K[b]),
            "XV": asf(V[b]),
            "WQ": asf(Wq[:, sl]),
            "WK": asf(Wk[:, sl]),
            "WV": asf(Wv[:, sl]),
            "WO": asf(Wo[sl, :]),
            "BQ": asf(bq[sl]).reshape(1, DH),
            "BK": asf(bk[sl]).reshape(1, DH),
            "BV": asf(bv[sl]).reshape(1, DH),
        })
    return in_maps


def combine(results, bo):
    bo = np.asarray(bo, dtype=np.float32)
    return np.stack([
        results[2 * b]["OUT"] + results[2 * b + 1]["OUT"] + bo
        for b in range(B)
    ])


def kernel(Q, K, V, Wq, bq, Wk, bk, Wv, bv, Wo, bo):
    nc = _get_nc(1)
    in_maps = make_in_maps(Q, K, V, Wq, bq, Wk, bk, Wv, bv, Wo, bo)
    res = run_bass_kernel_spmd(nc, in_maps, core_ids=list(range(8)))
    return combine(res.results, bo)



# revision 11
# speedup vs baseline: 1.3316x; 1.3316x over previous
"""Trainium2 Bass kernel for nn_MultiHeadAttention_3839700762945.

Full-shape contract: kernel(**inputs) takes the unsharded numpy inputs and
returns the full [4, 2048, 1024] output.

Sharding (8 cores): core c handles (batch b = c//2, head-half = c%2).
Each core computes q/k/v projections for its 8 heads (512 of the 1024 dim
columns) over the full sequence, runs attention for those heads, and emits a
partial output projection  OT_half.T @ Wo[half]  of shape [2048, 1024].
Host combines: out[b] = partial[2b] + partial[2b+1] + bo.  No collectives.

Key design points (vs the earlier staged kernel):
  - Host pre-transposes Q/K/V to [dim, seq] and pre-rounds everything to
    bf16, so the kernel does zero PE transposes and zero dtype-convert
    copies; all matmuls run at full bf16 rate.
  - Scores matmuls for a head PAIR run concurrently via PE row tiling
    (K=64 each, tile_position rows 0-63 / 64-127), halving scores PE time.
  - Softmax denominator rides as a 65th "ones" row of the AV stationary;
    normalization = DVE reciprocal + GPSIMD partition_broadcast + DVE mul.
  - Emission order is slot-scheduled: projection work (K/V/Q units) is
    interleaved between attention pair-groups so ScalarE (exp, the ~265us
    wall at 1 elem/lane/cycle) starts ~10us in and rarely starves.
"""

import sys

for _p in ("/opt/trn_rl_repo", "/opt/pypackages"):
    if _p not in sys.path:
        sys.path.insert(0, _p)

import numpy as np
import ml_dtypes

import concourse.bass as bass
import concourse.mybir as mybir
import concourse.tile as tile
import concourse.bacc as bacc

F32 = mybir.dt.float32
BF16 = mybir.dt.bfloat16
AF = mybir.ActivationFunctionType
BF = ml_dtypes.bfloat16

B, S, DIM = 4, 2048, 1024
DH = 512          # dim columns per core (8 heads x 64)
NH = 8            # heads per core
HD = 64
P = 128
NKC = DIM // P    # 8 contraction chunks for projections
NMC = DH // P     # 4 output-dim chunks
NSK = S // P      # 16 sk chunks
SQT = 512         # attention query tile
NSQT = S // SQT   # 4
EG = 2            # sk chunks per exp group
NG = NSK // EG    # 8 groups per head
NPAIR = NH // 2   # 4 head pairs
INV_SQRT_HD = 0.125
PT_BUFS = 3       # P-tile pipeline depth (pair-groups in flight)


def build_nc(reps: int = 1, mode: str = "full"):
    nc = bacc.Bacc("TRN2", target_bir_lowering=False, debug=False, num_devices=8)

    XQT = nc.dram_tensor("XQT", (DIM, S), BF16, kind="ExternalInput")
    XKT = nc.dram_tensor("XKT", (DIM, S), BF16, kind="ExternalInput")
    XVT = nc.dram_tensor("XVT", (DIM, S), BF16, kind="ExternalInput")
    WQ = nc.dram_tensor("WQ", (DIM, DH), BF16, kind="ExternalInput")
    WK = nc.dram_tensor("WK", (DIM, DH), BF16, kind="ExternalInput")
    WV = nc.dram_tensor("WV", (DIM, DH), BF16, kind="ExternalInput")
    WO = nc.dram_tensor("WO", (DH, DIM), BF16, kind="ExternalInput")
    BQ = nc.dram_tensor("BQ", (1, DH), BF16, kind="ExternalInput")
    BK = nc.dram_tensor("BK", (1, DH), BF16, kind="ExternalInput")
    BV = nc.dram_tensor("BV", (1, DH), BF16, kind="ExternalInput")
    OUT = nc.dram_tensor("OUT", (S, DIM), F32, kind="ExternalOutput")

    with tile.TileContext(nc) as tc:
        with (
            tc.tile_pool(name="persist", bufs=1) as pc,
            tc.tile_pool(name="xstage", bufs=2) as px,
            tc.tile_pool(name="work", bufs=2) as p2,
            tc.tile_pool(name="ptile", bufs=PT_BUFS) as p4,
            tc.tile_pool(name="ps_sc", bufs=2, space="PSUM") as ps_sc,
            tc.tile_pool(name="ps_av", bufs=2, space="PSUM") as ps_av,
            tc.tile_pool(name="ps_pp", bufs=2, space="PSUM") as ps_pp,
        ):
            pools = dict(pc=pc, px=px, p2=p2, p4=p4,
                         ps_sc=ps_sc, ps_av=ps_av, ps_pp=ps_pp)
            drams = dict(XQT=XQT, XKT=XKT, XVT=XVT, WQ=WQ, WK=WK, WV=WV,
                         WO=WO, BQ=BQ, BK=BK, BV=BV, OUT=OUT)
            for _rep in range(reps):
                _emit_rep(nc, pools, drams, mode)

    nc.compile()
    return nc


def _emit_rep(nc, pools, drams, mode):
    pc, px, p2, p4 = pools["pc"], pools["px"], pools["p2"], pools["p4"]
    ps_sc, ps_av, ps_pp = pools["ps_sc"], pools["ps_av"], pools["ps_pp"]

    # ---- constants / persistent tiles -------------------------------------
    ones = pc.tile([1, SQT], BF16, tag="ones")
    nc.vector.memset(ones[:], 1.0)

    # weights: [p(dim within chunk), kc, dh-cols]
    wsb = {}
    for nm, W in (("k", drams["WK"]), ("q", drams["WQ"]), ("v", drams["WV"])):
        w = pc.tile([P, NKC, DH], BF16, tag=f"w{nm}")
        nc.sync.dma_start(w[:], W.ap().rearrange("(kc p) d -> p kc d", p=P))
        wsb[nm] = w
    wo_sb = pc.tile([P, NMC, DIM], BF16, tag="wo")
    nc.sync.dma_start(wo_sb[:], drams["WO"].ap().rearrange("(kc p) d -> p kc d", p=P))
    brow = {}
    for nm, Bd in (("k", drams["BK"]), ("q", drams["BQ"]), ("v", drams["BV"])):
        t = pc.tile([1, DH], BF16, tag=f"b{nm}")
        nc.sync.dma_start(t[:], Bd.ap())
        brow[nm] = t

    # X staging: K/V full [p, kc, s], DMA'd in 4 seq-slices; Q streamed per sqt
    xsb = {}
    for nm, X in (("k", drams["XKT"]), ("v", drams["XVT"])):
        x = px.tile([P, NKC, S], BF16, tag=f"x{nm}", bufs=1)
        xv = X.ap().rearrange("(kc p) s -> p kc s", p=P)
        for sb in range(4):
            sl = slice(sb * SQT, (sb + 1) * SQT)
            nc.sync.dma_start(x[:, :, sl], xv[:, :, sl])
        xsb[nm] = x
    xq_view = drams["XQT"].ap().rearrange("(kc p) s -> p kc s", p=P)
    xqs = {}

    def prefetch_xq(sqt):
        t = px.tile([P, NKC, SQT], BF16, tag="xq", bufs=2)
        nc.sync.dma_start(t[:], xq_view[:, :, sqt * SQT:(sqt + 1) * SQT])
        xqs[sqt] = t

    prefetch_xq(0)

    # persistent activations
    kT = pc.tile([P, NMC, S], BF16, tag="kT")
    qT = pc.tile([P, NMC, S], BF16, tag="qT")
    vsb = pc.tile([P, NSK, NH, HD + 2], BF16, tag="vsb")
    nc.vector.memset(vsb[:, :, :, HD:HD + 1], 1.0)

    # ---- filler units (projection work interleaved into attention slots) --
    def unit_kq(nm, dst, m, sb):
        """Project input `nm` chunk: dst[:, m, sb*512:(sb+1)*512]."""
        def go():
            psp = ps_pp.tile([P, SQT], F32, tag="pp")
            for k in range(NKC):
                rhs = (xqs[sb][:, k, :] if nm == "q"
                       else xsb[nm][:, k, sb * SQT:(sb + 1) * SQT])
                nc.tensor.matmul(
                    psp[:], wsb[nm][:, k, m * P:(m + 1) * P], rhs,
                    start=(k == 0), stop=False)
            nc.tensor.matmul(
                psp[:], brow[nm][0:1, m * P:(m + 1) * P], ones[0:1, :],
                start=False, stop=True)
            nc.vector.tensor_copy(dst[:, m, sb * SQT:(sb + 1) * SQT], psp[:])
        return go

    def unit_v(c):
        """Project v sk-chunk c -> vsb[:, c, :, 0:64]."""
        def go():
            psv = ps_pp.tile([P, DH], F32, tag="pp")
            for k in range(NKC):
                nc.tensor.matmul(
                    psv[:], xsb["v"][:, k, c * P:(c + 1) * P], wsb["v"][:, k, :],
                    start=(k == 0), stop=False)
            nc.tensor.matmul(
                psv[:], ones[0:1, 0:P], brow["v"][0:1, :],
                start=False, stop=True)
            nc.vector.tensor_copy(
                vsb[:, c, :, 0:HD],
                psv[:].rearrange("p (h d) -> p h d", h=NH))
        return go

    def unit_outproj(sqt, ot, m, n2):
        """OUT[sqt*512 + m*128 .. +128, n2*512:(n2+1)*512]."""
        def go():
            pso = ps_pp.tile([P, DH], F32, tag="pp")
            for k in range(NMC):
                nc.tensor.matmul(
                    pso[:], ot[:, k, m * P:(m + 1) * P],
                    wo_sb[:, k, n2 * DH:(n2 + 1) * DH],
                    start=(k == 0), stop=(k == NMC - 1))
            ostg = p2.tile([P, DH], F32, tag="ostg")
            nc.vector.tensor_copy(ostg[:], pso[:])
            r0 = sqt * SQT + m * P
            nc.sync.dma_start(
                drams["OUT"].ap()[r0:r0 + P, n2 * DH:(n2 + 1) * DH], ostg[:])
        return go

    # filler queue with readiness bookkeeping
    emitted = {"k": set(), "q": set(), "v": set()}

    def mk_kq(nm, dst, m, sb):
        u = unit_kq(nm, dst, m, sb)
        def go(u=u, nm=nm, m=m, sb=sb):
            u()
            emitted[nm].add((m, sb))
        return go

    def mk_v(c):
        u = unit_v(c)
        def go(u=u, c=c):
            u()
            emitted["v"].add(c)
        return go

    fillers = []
    # order: per pair p: Q0(p) then K(p, 0..3); V chunks woven two per pair
    vq = list(range(NSK))
    for p in range(NPAIR):
        fillers.append(mk_kq("q", qT, p, 0))
        for sb in range(4):
            fillers.append(mk_kq("k", kT, p, sb))
            if vq and (p > 0 or sb > 1):
                fillers.append(mk_v(vq.pop(0)))
    while vq:
        fillers.append(mk_v(vq.pop(0)))

    def need(nm, m, sb):
        while (m, sb) not in emitted[nm]:
            assert fillers, f"filler queue empty but need {nm} {(m, sb)}"
            fillers.pop(0)()

    def need_v(c):
        while c not in emitted["v"]:
            assert fillers, f"filler queue empty but need v chunk {c}"
            fillers.pop(0)()

    # prelude: first K/Q units so scores can start early
    fillers.pop(0)()   # Q0(0)
    fillers.pop(0)()   # K(0, 0)

    # ---- attention --------------------------------------------------------
    def emit_scores(pair, g, sqt):
        """Returns the bf16 P tile [128, 2, EG, SQT] for this pair-group."""
        sq0 = sqt * SQT
        need("k", pair, (g * EG) // 4)
        need("k", pair, (g * EG + EG - 1) // 4)
        need("q", pair, sqt)
        ptt = p4.tile([P, 2, EG, SQT], BF16, tag="pt")
        for half in range(2):
            pss = ps_sc.tile([P, EG, SQT], F32, tag="sc")
            b0 = half * HD
            for ci in range(EG):
                c = g * EG + ci
                nc.tensor.matmul(
                    pss[:, ci, :],
                    kT[b0:b0 + HD, pair, c * P:(c + 1) * P],
                    qT[b0:b0 + HD, pair, sq0:sq0 + SQT],
                    start=True, stop=True,
                    tile_position=(b0, 0))
            nc.scalar.activation(ptt[:, half, :, :], pss[:], AF.Exp,
                                 scale=INV_SQRT_HD)
        return ptt

    def emit_av(pair, g, ptt, psavs):
        for half in range(2):
            h = pair * 2 + half
            if g == 0:
                psavs[half] = ps_av.tile([P, SQT], F32, tag="av", name="psav")
            for ci in range(EG):
                c = g * EG + ci
                need_v(c)
                nc.tensor.matmul(
                    psavs[half][0:HD + 1, :],
                    vsb[:, c, h, 0:HD + 1],
                    ptt[:, half, ci, :],
                    start=(c == 0), stop=(c == NSK - 1))

    def emit_norm(pair, ot, psavs):
        for half in range(2):
            h = pair * 2 + half
            psav = psavs[half]
            # reciprocal_approx_fast reads garbage from a PSUM source; stage
            # the row-sum into SBUF first.
            rsh = p2.tile([1, SQT], F32, tag="rsh")
            rst = p2.tile([1, SQT], F32, tag="rst")
            nc.vector.tensor_copy(rst[:], psav[HD:HD + 1, :])
            nc.vector.reciprocal_approx_fast(rsh[:], rst[:])
            bcs = p2.tile([HD, SQT], F32, tag="bcs")
            if "bpe" in mode:
                rshr = p2.tile([1, SQT], BF16, tag="rshr")
                nc.vector.tensor_copy(rshr[:], rsh[:])
                psb = ps_pp.tile([P, SQT], F32, tag="pp", name="psb")
                nc.tensor.matmul(psb[0:HD, :], ones[0:1, 0:HD], rshr[:],
                                 start=True, stop=True)
                nc.vector.tensor_copy(bcs[:], psb[0:HD, :])
            else:
                nc.gpsimd.partition_broadcast(bcs[:], rsh[:], channels=HD)
            base = half * HD
            mch = pair
            nc.vector.tensor_mul(ot[base:base + HD, mch, :],
                                 psav[0:HD, :], bcs[:])

    # software pipeline over (sqt, pair, group); AV lags scores by 1 group.
    ot_prev = None
    sqt_prev = None
    for sqt in range(NSQT):
        if sqt + 1 < NSQT:
            prefetch_xq(sqt + 1)
        ot = p2.tile([P, NMC, SQT], BF16, tag="ot")
        for pair in range(NPAIR):
            psavs = {}
            ptts = {}
            for g in range(NG + 1):
                if g < NG:
                    ptts[g] = emit_scores(pair, g, sqt)
                    # one filler slot per group keeps PE fed while ACT drains
                    if fillers:
                        fillers.pop(0)()
                if g >= 1:
                    emit_av(pair, g - 1, ptts.pop(g - 1), psavs)
            emit_norm(pair, ot, psavs)
            # spread previous sq-tile's output projection over this tile:
            # pair p emits tiles (m=p, n2=0..1) -> 8 tiles per sq-tile
            if ot_prev is not None:
                for n2 in range(2):
                    unit_outproj(sqt_prev, ot_prev, pair, n2)()
            # next sq-tile's Q projection, one m-chunk per pair
            if sqt + 1 < NSQT:
                mk_kq("q", qT, pair, sqt + 1)()
        ot_prev, sqt_prev = ot, sqt
    # drain remaining fillers and the last out-projection
    while fillers:
        fillers.pop(0)()
    for m in range(NMC):
        for n2 in range(2):
            unit_outproj(sqt_prev, ot_prev, m, n2)()


_cached = {}


def _get_nc(reps: int = 1, mode: str = "full"):
    key = (reps, mode)
    if key not in _cached:
        _cached[key] = build_nc(reps, mode)
    return _cached[key]


def _bf(x):
    return np.ascontiguousarray(np.asarray(x, np.float32).astype(BF))


def make_in_maps(Q, K, V, Wq, bq, Wk, bk, Wv, bv, Wo, bo):
    xqt = [_bf(np.asarray(Q[b], np.float32).T) for b in range(B)]
    xkt = [_bf(np.asarray(K[b], np.float32).T) for b in range(B)]
    xvt = [_bf(np.asarray(V[b], np.float32).T) for b in range(B)]
    halves = []
    for half in range(2):
        sl = slice(half * DH, (half + 1) * DH)
        halves.append({
            "WQ": _bf(np.asarray(Wq)[:, sl]),
            "WK": _bf(np.asarray(Wk)[:, sl]),
            "WV": _bf(np.asarray(Wv)[:, sl]),
            "WO": _bf(np.asarray(Wo)[sl, :]),
            "BQ": _bf(np.asarray(bq)[sl]).reshape(1, DH),
            "BK": _bf(np.asarray(bk)[sl]).reshape(1, DH),
            "BV": _bf(np.asarray(bv)[sl]).reshape(1, DH),
        })
    in_maps = []
    for c in range(8):
        b, half = divmod(c, 2)
        m = {"XQT": xqt[b], "XKT": xkt[b], "XVT": xvt[b]}
        m.update(halves[half])
        in_maps.append(m)
    return in_maps


def combine(results, bo):
    bo = np.asarray(bo, dtype=np.float32)
    return np.stack([
        results[2 * b]["OUT"] + results[2 * b + 1]["OUT"] + bo
        for b in range(B)
    ])


def kernel(Q, K, V, Wq, bq, Wk, bk, Wv, bv, Wo, bo):
    from concourse.bass_utils import run_bass_kernel_spmd
    nc = _get_nc(1)
    in_maps = make_in_maps(Q, K, V, Wq, bq, Wk, bk, Wv, bv, Wo, bo)
    res = run_bass_kernel_spmd(nc, in_maps, core_ids=list(range(8)))
    return combine(res.results, bo)


# revision 17
# speedup vs baseline: 5.6987x; 4.2796x over previous
"""Trainium2 Bass kernel for nn_MultiHeadAttention_3839700762945.

Full-shape contract: kernel(**inputs) takes the unsharded numpy inputs and
returns the full [4, 2048, 1024] output.

Sharding (8 cores): core c handles (batch b = c//2, head-half = c%2).
Each core computes q/k/v projections for its 8 heads (512 of the 1024 dim
columns) over the full sequence, runs attention for those heads, and emits a
partial output projection  OT_half.T @ Wo[half]  of shape [2048, 1024].
Host combines: out[b] = partial[2b] + partial[2b+1] + bo.  No collectives.

Key design points (vs the earlier staged kernel):
  - Host pre-transposes Q/K/V to [dim, seq] and pre-rounds everything to
    bf16, so the kernel does zero PE transposes and zero dtype-convert
    copies; all matmuls run at full bf16 rate.
  - Scores matmuls for a head PAIR run concurrently via PE row tiling
    (K=64 each, tile_position rows 0-63 / 64-127), halving scores PE time.
  - Softmax denominator rides as a 65th "ones" row of the AV stationary;
    normalization = DVE reciprocal + GPSIMD partition_broadcast + DVE mul.
  - Emission order is slot-scheduled: projection work (K/V/Q units) is
    interleaved between attention pair-groups so ScalarE (exp, the ~265us
    wall at 1 elem/lane/cycle) starts ~10us in and rarely starves.
"""

import sys

for _p in ("/opt/trn_rl_repo", "/opt/pypackages"):
    if _p not in sys.path:
        sys.path.insert(0, _p)

import numpy as np
import ml_dtypes

import concourse.bass as bass
import concourse.mybir as mybir
import concourse.tile as tile
import concourse.bacc as bacc

F32 = mybir.dt.float32
BF16 = mybir.dt.bfloat16
AF = mybir.ActivationFunctionType
BF = ml_dtypes.bfloat16

B, S, DIM = 4, 2048, 1024
DH = 512          # dim columns per core (8 heads x 64)
NH = 8            # heads per core
HD = 64
P = 128
NKC = DIM // P    # 8 contraction chunks for projections
NMC = DH // P     # 4 output-dim chunks
NSK = S // P      # 16 sk chunks
SQT = 512         # attention query tile
NSQT = S // SQT   # 4
EG = 2            # sk chunks per exp group
NG = NSK // EG    # 8 groups per head
NPAIR = NH // 2   # 4 head pairs
INV_SQRT_HD = 0.125
PT_BUFS = 4       # P-tile pipeline depth (pair-groups in flight)


def build_nc(reps: int = 1, mode: str = "full"):
    nc = bacc.Bacc("TRN2", target_bir_lowering=False, debug=False, num_devices=8)

    XQT = nc.dram_tensor("XQT", (DIM, S), BF16, kind="ExternalInput")
    XKT = nc.dram_tensor("XKT", (DIM, S), BF16, kind="ExternalInput")
    XVT = nc.dram_tensor("XVT", (DIM, S), BF16, kind="ExternalInput")
    WQ = nc.dram_tensor("WQ", (DIM, DH), BF16, kind="ExternalInput")
    WK = nc.dram_tensor("WK", (DIM, DH), BF16, kind="ExternalInput")
    WV = nc.dram_tensor("WV", (DIM, DH), BF16, kind="ExternalInput")
    WO = nc.dram_tensor("WO", (DH, DIM), BF16, kind="ExternalInput")
    BQ = nc.dram_tensor("BQ", (1, DH), BF16, kind="ExternalInput")
    BK = nc.dram_tensor("BK", (1, DH), BF16, kind="ExternalInput")
    BV = nc.dram_tensor("BV", (1, DH), BF16, kind="ExternalInput")
    OUT = nc.dram_tensor("OUT", (S, DIM), F32, kind="ExternalOutput")

    with tile.TileContext(nc) as tc:
        with (
            tc.tile_pool(name="persist", bufs=1) as pc,
            tc.tile_pool(name="xstage", bufs=2) as px,
            tc.tile_pool(name="work", bufs=2) as p2,
            tc.tile_pool(name="ptile", bufs=PT_BUFS) as p4,
            tc.tile_pool(name="ps_sc", bufs=2, space="PSUM") as ps_sc,
            tc.tile_pool(name="ps_av", bufs=2, space="PSUM") as ps_av,
            tc.tile_pool(name="ps_pj", bufs=1, space="PSUM") as ps_pj,
            tc.tile_pool(name="ps_po", bufs=1, space="PSUM") as ps_po,
        ):
            pools = dict(pc=pc, px=px, p2=p2, p4=p4,
                         ps_sc=ps_sc, ps_av=ps_av, ps_pj=ps_pj, ps_po=ps_po)
            drams = dict(XQT=XQT, XKT=XKT, XVT=XVT, WQ=WQ, WK=WK, WV=WV,
                         WO=WO, BQ=BQ, BK=BK, BV=BV, OUT=OUT)
            for _rep in range(reps):
                _emit_rep(nc, pools, drams, mode)

    nc.compile()
    return nc


def _emit_rep(nc, pools, drams, mode):
    pc, px, p2, p4 = pools["pc"], pools["px"], pools["p2"], pools["p4"]
    ps_sc, ps_av = pools["ps_sc"], pools["ps_av"]
    ps_pj, ps_po = pools["ps_pj"], pools["ps_po"]

    # ---- constants / persistent tiles -------------------------------------
    ones = pc.tile([1, SQT], BF16, tag="ones")
    nc.vector.memset(ones[:], 1.0)

    # DMAs ordered by first use: K-path first so scores start early.
    wsb, brow, xsb = {}, {}, {}

    def dma_w(nm, W):
        w = pc.tile([P, NKC, DH], BF16, tag=f"w{nm}", name="w")
        nc.sync.dma_start(w[:], W.ap().rearrange("(kc p) d -> p kc d", p=P))
        wsb[nm] = w

    def dma_b(nm, Bd):
        t = pc.tile([1, DH], BF16, tag=f"b{nm}", name="t")
        nc.sync.dma_start(t[:], Bd.ap())
        brow[nm] = t

    def dma_x(nm, X):
        x = px.tile([P, NKC, S], BF16, tag=f"x{nm}", bufs=1, name="x")
        xsb[nm] = (x, X.ap().rearrange("(kc p) s -> p kc s", p=P))

    def dma_x_slice(nm, sb):
        x, xv = xsb[nm]
        sl = slice(sb * SQT, (sb + 1) * SQT)
        nc.sync.dma_start(x[:, :, sl], xv[:, :, sl])

    xq_view = drams["XQT"].ap().rearrange("(kc p) s -> p kc s", p=P)
    xqs = {}

    def prefetch_xq(sqt):
        t = px.tile([P, NKC, SQT], BF16, tag="xq", bufs=2)
        nc.sync.dma_start(t[:], xq_view[:, :, sqt * SQT:(sqt + 1) * SQT])
        xqs[sqt] = t

    dma_x("k", drams["XKT"])
    dma_x("v", drams["XVT"])
    dma_w("k", drams["WK"])
    dma_b("k", drams["BK"])
    dma_x_slice("k", 0)
    dma_w("q", drams["WQ"])
    dma_b("q", drams["BQ"])
    prefetch_xq(0)
    dma_w("v", drams["WV"])
    dma_b("v", drams["BV"])
    dma_x_slice("v", 0)
    dma_x_slice("k", 1)
    dma_x_slice("v", 1)
    dma_x_slice("k", 2)
    dma_x_slice("v", 2)
    dma_x_slice("k", 3)
    dma_x_slice("v", 3)
    wo_sb = pc.tile([P, NMC, DIM], BF16, tag="wo")
    nc.sync.dma_start(wo_sb[:], drams["WO"].ap().rearrange("(kc p) d -> p kc d", p=P))
    xsb = {nm: x for nm, (x, _) in xsb.items()}

    # persistent activations
    kT = pc.tile([P, NMC, S], BF16, tag="kT")
    qT = pc.tile([P, NMC, S], BF16, tag="qT")
    vsb = pc.tile([P, NSK, NH, HD + 2], BF16, tag="vsb")
    nc.vector.memset(vsb[:, :, :, HD:HD + 1], 1.0)

    # ---- filler units (projection work interleaved into attention slots) --
    # Each unit is split into two halves (~4 matmuls each) so one filler
    # slot never delays the next scores group by much more than the PE
    # slack inside a ScalarE-paced slot.
    emitted = {"k": set(), "q": set(), "v": set()}

    def mk_kq(nm, dst, m, sb):
        """Project input `nm` chunk: dst[:, m, sb*512:(sb+1)*512]."""
        st = {}

        def goA():
            psp = ps_pj.tile([P, SQT], F32, tag="pj", name="psp")
            st["t"] = psp
            for k in range(4):
                rhs = (xqs[sb][:, k, :] if nm == "q"
                       else xsb[nm][:, k, sb * SQT:(sb + 1) * SQT])
                nc.tensor.matmul(
                    psp[:], wsb[nm][:, k, m * P:(m + 1) * P], rhs,
                    start=(k == 0), stop=False)

        def goB():
            psp = st["t"]
            for k in range(4, NKC):
                rhs = (xqs[sb][:, k, :] if nm == "q"
                       else xsb[nm][:, k, sb * SQT:(sb + 1) * SQT])
                nc.tensor.matmul(
                    psp[:], wsb[nm][:, k, m * P:(m + 1) * P], rhs,
                    start=False, stop=False)
            nc.tensor.matmul(
                psp[:], brow[nm][0:1, m * P:(m + 1) * P], ones[0:1, :],
                start=False, stop=True)
            nc.vector.tensor_copy(dst[:, m, sb * SQT:(sb + 1) * SQT], psp[:])
            emitted[nm].add((m, sb))

        return [goA, goB]

    def mk_v(c):
        """Project v sk-chunk c -> vsb[:, c, :, 0:64]."""
        st = {}

        def goA():
            psv = ps_pj.tile([P, DH], F32, tag="pj", name="psv")
            st["t"] = psv
            for k in range(4):
                nc.tensor.matmul(
                    psv[:], xsb["v"][:, k, c * P:(c + 1) * P], wsb["v"][:, k, :],
                    start=(k == 0), stop=False)

        def goB():
            psv = st["t"]
            for k in range(4, NKC):
                nc.tensor.matmul(
                    psv[:], xsb["v"][:, k, c * P:(c + 1) * P], wsb["v"][:, k, :],
                    start=False, stop=False)
            nc.tensor.matmul(
                psv[:], ones[0:1, 0:P], brow["v"][0:1, :],
                start=False, stop=True)
            nc.vector.tensor_copy(
                vsb[:, c, :, 0:HD],
                psv[:].rearrange("p (h d) -> p h d", h=NH))
            emitted["v"].add(c)

        return [goA, goB]

    def unit_outproj(sqt, ot, m, n2):
        """OUT[sqt*512 + m*128 .. +128, n2*512:(n2+1)*512]."""
        def go():
            pso = ps_po.tile([P, DH], F32, tag="po", name="pso")
            for k in range(NMC):
                nc.tensor.matmul(
                    pso[:], ot[:, k, m * P:(m + 1) * P],
                    wo_sb[:, k, n2 * DH:(n2 + 1) * DH],
                    start=(k == 0), stop=(k == NMC - 1))
            ostg = p2.tile([P, DH], F32, tag="ostg")
            nc.vector.tensor_copy(ostg[:], pso[:])
            r0 = sqt * SQT + m * P
            nc.sync.dma_start(
                drams["OUT"].ap()[r0:r0 + P, n2 * DH:(n2 + 1) * DH], ostg[:])
        return go

    fillers = []
    # order: per pair p: Q0(p) then K(p, 0..3); V chunks woven two per pair
    vq = list(range(NSK))
    for p in range(NPAIR):
        fillers.extend(mk_kq("q", qT, p, 0))
        for sb in range(4):
            fillers.extend(mk_kq("k", kT, p, sb))
            if vq and (p > 0 or sb > 1):
                fillers.extend(mk_v(vq.pop(0)))
    while vq:
        fillers.extend(mk_v(vq.pop(0)))

    def need(nm, m, sb):
        while (m, sb) not in emitted[nm]:
            assert fillers, f"filler queue empty but need {nm} {(m, sb)}"
            fillers.pop(0)()

    def need_v(c):
        while c not in emitted["v"]:
            assert fillers, f"filler queue empty but need v chunk {c}"
            fillers.pop(0)()

    # prelude: first K/Q units so scores can start early
    need("q", 0, 0)
    need("k", 0, 0)

    # ---- attention --------------------------------------------------------
    def emit_scores(pair, g, sqt):
        """Returns the bf16 P tile [128, 2, EG, SQT] for this pair-group."""
        sq0 = sqt * SQT
        need("k", pair, (g * EG) // 4)
        need("k", pair, (g * EG + EG - 1) // 4)
        need("q", pair, sqt)
        ptt = p4.tile([P, 2, EG, SQT], BF16, tag="pt")
        for half in range(2):
            pss = ps_sc.tile([P, EG, SQT], F32, tag="sc")
            b0 = half * HD
            for ci in range(EG):
                c = g * EG + ci
                nc.tensor.matmul(
                    pss[:, ci, :],
                    kT[b0:b0 + HD, pair, c * P:(c + 1) * P],
                    qT[b0:b0 + HD, pair, sq0:sq0 + SQT],
                    start=True, stop=True,
                    tile_position=(b0, 0))
            nc.scalar.activation(ptt[:, half, :, :], pss[:], AF.Exp,
                                 scale=INV_SQRT_HD)
        return ptt

    def emit_av(pair, g, ptt, psavs):
        for half in range(2):
            h = pair * 2 + half
            if g == 0:
                psavs[half] = ps_av.tile([P, SQT], F32, tag="av", name="psav")
            for ci in range(EG):
                c = g * EG + ci
                need_v(c)
                nc.tensor.matmul(
                    psavs[half][0:HD + 1, :],
                    vsb[:, c, h, 0:HD + 1],
                    ptt[:, half, ci, :],
                    start=(c == 0), stop=(c == NSK - 1))

    def emit_norm(pair, ot, psavs):
        for half in range(2):
            h = pair * 2 + half
            psav = psavs[half]
            # reciprocal_approx_fast reads garbage from a PSUM source; stage
            # the row-sum into SBUF first.
            rsh = p2.tile([1, SQT], F32, tag="rsh")
            rst = p2.tile([1, SQT], F32, tag="rst")
            nc.vector.tensor_copy(rst[:], psav[HD:HD + 1, :])
            nc.vector.reciprocal_approx_fast(rsh[:], rst[:])
            bcs = p2.tile([HD, SQT], F32, tag="bcs")
            nc.gpsimd.partition_broadcast(bcs[:], rsh[:], channels=HD)
            base = half * HD
            mch = pair
            nc.vector.tensor_mul(ot[base:base + HD, mch, :],
                                 psav[0:HD, :], bcs[:])

    # software pipeline over (sqt, pair, group); AV lags scores by 1 group.
    ot_prev = None
    sqt_prev = None
    for sqt in range(NSQT):
        if sqt + 1 < NSQT:
            prefetch_xq(sqt + 1)
        ot = p2.tile([P, NMC, SQT], BF16, tag="ot")
        for pair in range(NPAIR):
            psavs = {}
            ptts = {}
            for g in range(NG + 1):
                if g < NG:
                    ptts[g] = emit_scores(pair, g, sqt)
                    # one filler slot per group keeps PE fed while ACT drains
                    if fillers:
                        fillers.pop(0)()
                if g >= 1:
                    emit_av(pair, g - 1, ptts.pop(g - 1), psavs)
            emit_norm(pair, ot, psavs)
            # spread previous sq-tile's output projection over this tile:
            # pair p emits tiles (m=p, n2=0..1) -> 8 tiles per sq-tile
            if ot_prev is not None:
                for n2 in range(2):
                    unit_outproj(sqt_prev, ot_prev, pair, n2)()
            # next sq-tile's Q projection, one m-chunk per pair
            if sqt + 1 < NSQT:
                for half_fn in mk_kq("q", qT, pair, sqt + 1):
                    half_fn()
        ot_prev, sqt_prev = ot, sqt
    # drain remaining fillers and the last out-projection
    while fillers:
        fillers.pop(0)()
    for m in range(NMC):
        for n2 in range(2):
            unit_outproj(sqt_prev, ot_prev, m, n2)()


_cached = {}


def _get_nc(reps: int = 1, mode: str = "full"):
    key = (reps, mode)
    if key not in _cached:
        _cached[key] = build_nc(reps, mode)
    return _cached[key]


def _bf(x):
    return np.ascontiguousarray(np.asarray(x, np.float32).astype(BF))


def make_in_maps(Q, K, V, Wq, bq, Wk, bk, Wv, bv, Wo, bo):
    xqt = [_bf(np.asarray(Q[b], np.float32).T) for b in range(B)]
    xkt = [_bf(np.asarray(K[b], np.float32).T) for b in range(B)]
    xvt = [_bf(np.asarray(V[b], np.float32).T) for b in range(B)]
    halves = []
    for half in range(2):
        sl = slice(half * DH, (half + 1) * DH)
        halves.append({
            "WQ": _bf(np.asarray(Wq)[:, sl]),
            "WK": _bf(np.asarray(Wk)[:, sl]),
            "WV": _bf(np.asarray(Wv)[:, sl]),
            "WO": _bf(np.asarray(Wo)[sl, :]),
            "BQ": _bf(np.asarray(bq)[sl]).reshape(1, DH),
            "BK": _bf(np.asarray(bk)[sl]).reshape(1, DH),
            "BV": _bf(np.asarray(bv)[sl]).reshape(1, DH),
        })
    in_maps = []
    for c in range(8):
        b, half = divmod(c, 2)
        m = {"XQT": xqt[b], "XKT": xkt[b], "XVT": xvt[b]}
        m.update(halves[half])
        in_maps.append(m)
    return in_maps


def combine(results, bo):
    bo = np.asarray(bo, dtype=np.float32)
    return np.stack([
        results[2 * b]["OUT"] + results[2 * b + 1]["OUT"] + bo
        for b in range(B)
    ])


def kernel(Q, K, V, Wq, bq, Wk, bk, Wv, bv, Wo, bo):
    from concourse.bass_utils import run_bass_kernel_spmd
    nc = _get_nc(1)
    in_maps = make_in_maps(Q, K, V, Wq, bq, Wk, bk, Wv, bv, Wo, bo)
    res = run_bass_kernel_spmd(nc, in_maps, core_ids=list(range(8)))
    return combine(res.results, bo)
